# revision 41
# baseline (speedup 1.0000x reference)
"""Trainium2 Bass kernel for nn_Attention_19035295056566 (v2).

Dense transformer block with head-axis attention: per token a 6x6
softmax over the head axis; torch-faithful dim-mixing transpose; then
proj. Pack-16 formulation: 16 tokens x 6 heads = 96 rows per PE
matmul, g-major packing (pack row = g*16 + t), DMA-transpose (xbar)
for every layout move (no PE transposes, no DRAM bounce), block-diag
mask applied multiplicatively after exp.

Distribution: pure data-parallel over batch B=8 across 8 NeuronCores
(one batch element per core, weights replicated, no collectives).

Self-contained: hardcodes shapes B=8, N=4096, C=768, H=6, D=128.
"""

import sys

for _p in ("/opt/trn_rl_repo",):
    if _p not in sys.path:
        sys.path.insert(0, _p)

import numpy as np
import ml_dtypes

from concourse import bass, bacc, mybir, tile

F32 = mybir.dt.float32
BF16 = mybir.dt.bfloat16

B, N_TOK, C = 8, 4096, 768
H, D = 6, 128
SCALE = float(D) ** -0.5
NCH = C // 128  # 6 contraction chunks
GRP = 16  # tokens per pack group
PACK = GRP * H  # 96 pack rows


def build_graph(n_tok=N_TOK, chunk=512, reps=1, debug=False, bias_zero=False,
                gb=4, eng_rot="vsg", tsp_rot="vg", vq="y", pq="y", x0_split=True):
    """Single-core Bass graph (same graph runs SPMD on 8 cores).

    gb: groups per softmax batch (psum tile [96, gb, 96]).
    eng_rot: engine rotation for the QKV psum->sbuf copies.
    tsp_rot: engine rotation for the normalize tensor_scalar.
    vq/pq: queue for vpk/pts transposes ('y'=sync, 's'=scalar).
    x0_split: load chunk 0's x as 4 row-slab DMAs for earlier xT start.
    """
    nc = bacc.Bacc("TRN2", target_bir_lowering=False, debug=debug)

    x_d = nc.dram_tensor("x", [n_tok, C], F32, kind="ExternalInput")
    wqkv_d = nc.dram_tensor("w_qkv", [C, 3 * C], F32, kind="ExternalInput")
    wproj_d = nc.dram_tensor("w_proj", [C, C], F32, kind="ExternalInput")
    bproj_d = nc.dram_tensor("b_proj", [C], F32, kind="ExternalInput")
    m01_d = nc.dram_tensor("m01", [PACK, PACK], BF16, kind="ExternalInput")
    out_d = nc.dram_tensor("out", [n_tok, C], F32, kind="ExternalOutput")

    n_chunks = n_tok // chunk
    ng = chunk // GRP  # pack groups per chunk
    nb = ng // gb  # softmax batches per chunk
    nr = chunk // 128  # 128-token slabs per chunk
    n_tiles = n_tok // 128

    with tile.TileContext(nc) as tc:
        with (
            tc.tile_pool(name="const", bufs=1) as constp,
            tc.tile_pool(name="sb", bufs=2) as sbp,
            tc.tile_pool(name="small", bufs=3) as smallp,
            tc.tile_pool(name="outt", bufs=1) as outtp,
            tc.tile_pool(name="psq", bufs=2, space="PSUM") as psq,
            tc.tile_pool(name="psqk", bufs=2, space="PSUM") as psqk,
            tc.tile_pool(name="psav", bufs=2, space="PSUM") as psav,
            tc.tile_pool(name="pspj", bufs=2, space="PSUM") as pspj,
        ):
            # ---- constants. Casting DMAs are gpsimd-only; x chunk 0 is
            # issued first inside _run_body, weights pace the first QKV
            # accumulation chain, wproj (needed only at proj) goes last.
            wqkv_sb = constp.tile([128, NCH, 3 * C], BF16)
            wproj_sb = constp.tile([128, NCH, C], BF16)
            m01_sb = constp.tile([PACK, PACK], BF16)
            nc.sync.dma_start(out=m01_sb[:], in_=m01_d[:])

            def load_consts():
                # big DMAs (one per q/k/v third): fewer descriptors -> no
                # semaphore-pool pacing stalls
                wqkv_r = wqkv_d[:].rearrange("(ch p) c -> p ch c", p=128)
                for s in range(3):
                    nc.gpsimd.dma_start(
                        out=wqkv_sb[:, :, C * s : C * (s + 1)],
                        in_=wqkv_r[:, :, C * s : C * (s + 1)],
                    )

            def load_wproj():
                nc.gpsimd.dma_start(
                    out=wproj_sb[:],
                    in_=wproj_d[:].rearrange("(j p) c -> p j c", p=128),
                )
            if bias_zero:
                bias_sb = None
            else:
                bias_row = constp.tile([1, C], F32)
                nc.sync.dma_start(out=bias_row[:], in_=bproj_d.ap().unsqueeze(0))
                bias_sb = constp.tile([128, C], F32)
                nc.gpsimd.partition_broadcast(bias_sb[:], bias_row[:])

            # attention output, transposed/mixed: col h*n_tok + t holds
            # out_attn[t, h, :] over the 128 d-partitions. bf16.
            outT = outtp.tile([128, H * n_tok], BF16)

            engs = {"v": nc.vector, "s": nc.scalar, "g": nc.gpsimd}
            rot = [engs[e] for e in eng_rot]
            trot = [engs[e] for e in tsp_rot]

            rot0 = [engs[e] for e in "vs"]

            def copy_op(i, out, in_, first_chunk=False):
                # chunk 0 avoids Pool: its queue is busy with weight DMAs
                r = rot0 if first_chunk else rot
                e = r[i % len(r)]
                if e is nc.scalar:
                    e.copy(out, in_)
                else:
                    e.tensor_copy(out, in_)

            qmap = {"y": nc.sync, "s": nc.scalar}
            for _rep in range(reps):
                _run_body(
                    nc, n_tok, chunk, gb, copy_op, trot,
                    sbp, smallp, psq, psqk, psav, pspj,
                    wqkv_sb, wproj_sb, m01_sb, bias_sb, outT,
                    x_d, out_d,
                    load_consts if _rep == 0 else None,
                    load_wproj if _rep == 0 else None,
                    qmap[vq], qmap[pq], x0_split,
                )

    nc.compile()
    return nc


def _run_body(
    nc, n_tok, chunk, gb, copy_op, trot,
    sbp, smallp, psq, psqk, psav, pspj,
    wqkv_sb, wproj_sb, m01_sb, bias_sb, outT,
    x_d, out_d,
    load_consts=None, load_wproj=None,
    vq_eng=None, pq_eng=None, x0_split=True,
):
    vq_eng = vq_eng or nc.sync
    pq_eng = pq_eng or nc.sync
    n_chunks = n_tok // chunk
    ng = chunk // GRP
    nb = ng // gb
    nr = chunk // 128
    n_tiles = n_tok // 128

    outT_ht = outT[:].rearrange("p (h t) -> p h t", h=H)

    # deferred AV work: (vpk, pts, t0) per chunk, executed one chunk late
    # so the softmax chains have a full chunk of PE work to hide behind.
    pending = []

    def av_phase(vpk, pts, t0):
        for b in range(nb):
            av_ps = psav.tile([128, gb, PACK], F32, tag="av")
            for jj in range(gb):
                j = gb * b + jj
                nc.tensor.matmul(
                    av_ps[:, jj, :],
                    vpk[0:PACK, j, :],
                    pts[0:PACK, j, :],
                    start=True,
                    stop=True,
                )
            tb = t0 + GRP * gb * b
            dst = outT_ht[:, :, tb : tb + GRP * gb].rearrange(
                "p h (j t) -> p j h t", j=gb
            )
            nc.vector.tensor_copy(
                dst, av_ps[:].rearrange("p j (h t) -> p j h t", h=H)
            )

    def load_x(cc, split=False):
        # cast f32->bf16 during DMA; one DMA per chunk in steady state
        x_bf = sbp.tile([128, nr, C], BF16, tag="x_bf")
        if split:
            for rg in range(nr):
                nc.gpsimd.dma_start(
                    out=x_bf[:, rg, :],
                    in_=x_d[cc * chunk + 128 * rg : cc * chunk + 128 * (rg + 1), :],
                )
        else:
            src = x_d[cc * chunk : (cc + 1) * chunk, :].rearrange(
                "(rg p) c -> p rg c", p=128
            )
            nc.gpsimd.dma_start(out=x_bf[:], in_=src)
        return x_bf

    def make_xT(x_bf):
        xT = sbp.tile([128, NCH, chunk], BF16, tag="xT")
        for rg in range(nr):
            for ch in range(NCH):
                nc.sync.dma_start_transpose(
                    out=xT[:, ch, 128 * rg : 128 * (rg + 1)],
                    in_=x_bf[:, rg, 128 * ch : 128 * (ch + 1)],
                )
        return xT

    x_next = load_x(0, split=x0_split)
    xT_next = make_xT(x_next)
    if load_consts is not None:
        load_consts()

    for cc in range(n_chunks):
        t0 = cc * chunk
        xT = xT_next
        if cc == 1 and load_wproj is not None:
            load_wproj()

        # ---- q, k (g-major pack layout), v (group-padded) ----
        # q_il/k_il: [128 d, ng, 96], col inside group = h*16 + t
        # v_ilp:     [128 d, ng, 128], col = g*16 + t, cols 96:128 pad
        q_il = sbp.tile([128, ng, PACK], BF16, tag="q_il")
        k_il = sbp.tile([128, ng, PACK], BF16, tag="k_il")
        v_ilp = sbp.tile([128, ng, 128], BF16, tag="v_ilp")
        q_v = q_il[:].rearrange("p j (s t) -> p j s t", t=GRP)
        k_v = k_il[:].rearrange("p j (s t) -> p j s t", t=GRP)
        v_v = v_ilp[:].rearrange("p j (s t) -> p j s t", t=GRP)
        # pad slots 6,7 feed the xbar transpose (never compute) — zero for
        # the sim's uninit checker
        nc.gpsimd.memset(v_v[:, :, H:8, :], 0.0)
        for s in range(3):
            dst_v = (q_v, k_v, v_v)[s]
            for h in range(H):
                col0 = (s * H + h) * 128
                ps = psq.tile([128, chunk], F32, tag="qkv")
                for ch in range(NCH):
                    nc.tensor.matmul(
                        ps[:],
                        wqkv_sb[:, ch, col0 : col0 + 128],
                        xT[:, ch, :],
                        start=(ch == 0),
                        stop=(ch == NCH - 1),
                    )
                copy_op(
                    s * H + h,
                    dst_v[:, :, h, :],
                    ps[:].rearrange("p (j t) -> p j t", t=GRP),
                    first_chunk=(cc == 0 and load_consts is not None),
                )

        # prefetch next x only now: earlier would block Pool's psum-copy
        # stream behind a 2.4us cast DMA
        if cc + 1 < n_chunks:
            x_next = load_x(cc + 1)
            # next chunk's xT goes on SP *before* this chunk's vpk/pts so
            # it can't get stuck behind them in the queue
            xT_next = make_xT(x_next)

        # ---- V_pack via xbar DMA transpose: rows g*16+t (+pad), cols d ----
        vpk = sbp.tile([128, ng, 128], BF16, tag="vpk")
        for j in range(ng):
            vq_eng.dma_start_transpose(out=vpk[:, j, :], in_=v_ilp[:, j, :])

        # ---- AV for the PREVIOUS chunk, issued before this chunk's
        # softmax so its DVE copies aren't queued behind it ----
        if pending:
            av_phase(*pending.pop())

        # ---- QK^T + softmax + P^T per batch of gb groups ----
        pts = sbp.tile([128, ng, PACK], BF16, tag="pts")
        for b in range(nb):
            qk_ps = psqk.tile([PACK, gb, PACK], F32, tag="qk")
            for jj in range(gb):
                j = gb * b + jj
                nc.tensor.matmul(
                    qk_ps[:, jj, :],
                    q_il[:, j, :],
                    k_il[:, j, :],
                    start=True,
                    stop=True,
                )
            ex = smallp.tile([PACK, gb, PACK], BF16, tag="ex")
            nc.scalar.activation(
                ex[:], qk_ps[:], mybir.ActivationFunctionType.Exp, scale=SCALE
            )
            exm = smallp.tile([PACK, gb, PACK], BF16, tag="exm")
            nc.vector.tensor_tensor(
                exm[:],
                ex[:],
                m01_sb[:].unsqueeze(1).broadcast_to([PACK, gb, PACK]),
                op=mybir.AluOpType.mult,
            )
            zs = smallp.tile([PACK, gb], BF16, tag="zs")
            rc = smallp.tile([PACK, gb], F32, tag="rc")
            with nc.allow_low_precision(reason="softmax denom, 6 terms, bf16 ok"):
                nc.vector.tensor_reduce(
                    zs[:], exm[:], axis=mybir.AxisListType.X, op=mybir.AluOpType.add
                )
                nc.vector.reciprocal(rc[:], zs[:])
            # P padded to 128 cols for the xbar transpose; pad cols hold
            # garbage that lands in unused P^T rows 96:128.
            p_t = smallp.tile([PACK, gb, 128], BF16, tag="p_t")
            nc.gpsimd.memset(p_t[:, :, PACK:128], 0.0)
            for jj in range(gb):
                trot[jj % len(trot)].tensor_scalar(
                    p_t[:, jj, 0:PACK],
                    exm[:, jj, :],
                    rc[:, jj : jj + 1],
                    None,
                    op0=mybir.AluOpType.mult,
                )
            for jj in range(gb):
                j = gb * b + jj
                pq_eng.dma_start_transpose(
                    out=pts[:, j, :], in_=p_t[:, jj, :]
                )

        pending.append((vpk, pts, t0))

    # ---- proj: y[n', o] = sum_j OM[6n'+j] @ Wj (+ b) ----
    # Tiles whose outT columns avoid the last chunk's tokens are issued
    # BEFORE the final AV phase, hiding its softmax latency behind PE work.
    omT = outT[:].rearrange("p (i six) -> p i six", six=H)

    def tile_needs_last_chunk(t):
        return any(
            (col % n_tok) >= n_tok - chunk
            for col in range(768 * t, 768 * (t + 1))
        )

    early = [t for t in range(n_tiles) if not tile_needs_last_chunk(t)]
    late = [t for t in range(n_tiles) if tile_needs_last_chunk(t)]

    def proj_tile(t):
        ya = pspj.tile([128, 384], F32, tag="pj")
        yb = pspj.tile([128, 384], F32, tag="pj")
        for j in range(H):
            lhsT = omT[:, 128 * t : 128 * (t + 1), j]
            nc.tensor.matmul(
                ya[:], lhsT, wproj_sb[:, j, 0:384],
                start=(j == 0), stop=(j == H - 1),
            )
            nc.tensor.matmul(
                yb[:], lhsT, wproj_sb[:, j, 384:768],
                start=(j == 0), stop=(j == H - 1),
            )
        y_sb = smallp.tile([128, C], F32, tag="y_sb")
        if bias_sb is None:
            nc.vector.tensor_copy(y_sb[:, 0:384], ya[:])
            nc.scalar.copy(y_sb[:, 384:768], yb[:])
        else:
            nc.vector.scalar_tensor_tensor(
                y_sb[:, 0:384], ya[:], 1.0, bias_sb[:, 0:384],
                op0=mybir.AluOpType.mult, op1=mybir.AluOpType.add,
            )
            nc.vector.scalar_tensor_tensor(
                y_sb[:, 384:768], yb[:], 1.0, bias_sb[:, 384:768],
                op0=mybir.AluOpType.mult, op1=mybir.AluOpType.add,
            )
        nc.sync.dma_start(out=out_d[128 * t : 128 * (t + 1), :], in_=y_sb[:])

    for t in early:
        proj_tile(t)
    av_phase(*pending.pop())
    for t in late:
        proj_tile(t)


_CACHED = {}

BEST = dict()


def _get_graph(n_tok=N_TOK, chunk=512, **kw):
    key = (n_tok, chunk, tuple(sorted(kw.items())))
    if key not in _CACHED:
        _CACHED[key] = build_graph(n_tok, chunk, **kw)
    return _CACHED[key]


def _m01():
    return np.tile(np.eye(GRP, dtype=ml_dtypes.bfloat16), (H, H))


def make_in_map(x_i, w_qkv, w_proj, b_proj):
    return {
        "x": np.ascontiguousarray(x_i, dtype=np.float32),
        "w_qkv": np.asarray(w_qkv, dtype=np.float32),
        "w_proj": np.asarray(w_proj, dtype=np.float32),
        "b_proj": np.asarray(b_proj, dtype=np.float32),
        "m01": _m01(),
    }


def kernel(x, w_qkv, w_proj, b_proj):
    from concourse.bass_utils import run_bass_kernel_spmd

    x = np.asarray(x, dtype=np.float32)
    nc = _get_graph(
        bias_zero=bool(np.all(np.asarray(b_proj) == 0.0)), **BEST
    )
    in_maps = [make_in_map(x[i], w_qkv, w_proj, b_proj) for i in range(B)]
    res = run_bass_kernel_spmd(nc, in_maps, core_ids=list(range(B)))
    out = np.stack([res.results[i]["out"] for i in range(B)], axis=0)
    return out.astype(np.float32)


if __name__ == "__main__":
    rng = np.random.default_rng(0)
    x = rng.standard_normal((B, N_TOK, C), dtype=np.float32)
    w_qkv = (rng.standard_normal((C, 3 * C)) * C**-0.5).astype(np.float32)
    w_proj = (rng.standard_normal((C, C)) * C**-0.5).astype(np.float32)
    b_proj = np.zeros((C,), dtype=np.float32)
    y = kernel(x=x, w_qkv=w_qkv, w_proj=w_proj, b_proj=b_proj)
    print(y.shape, y.dtype)


# revision 45
# speedup vs baseline: 1552.4853x; 1552.4853x over previous
"""Trainium2 Bass kernel for nn_Attention_19035295056566 (v2).

Dense transformer block with head-axis attention: per token a 6x6
softmax over the head axis; torch-faithful dim-mixing transpose; then
proj. Pack-16 formulation: 16 tokens x 6 heads = 96 rows per PE
matmul, g-major packing (pack row = g*16 + t), DMA-transpose (xbar)
for every layout move (no PE transposes, no DRAM bounce), block-diag
mask applied multiplicatively after exp.

Distribution: pure data-parallel over batch B=8 across 8 NeuronCores
(one batch element per core, weights replicated, no collectives).

Self-contained: hardcodes shapes B=8, N=4096, C=768, H=6, D=128.
"""

import sys

for _p in ("/opt/trn_rl_repo",):
    if _p not in sys.path:
        sys.path.insert(0, _p)

import numpy as np
import ml_dtypes

from concourse import bass, bacc, mybir, tile

F32 = mybir.dt.float32
BF16 = mybir.dt.bfloat16

B, N_TOK, C = 8, 4096, 768
H, D = 6, 128
SCALE = float(D) ** -0.5
NCH = C // 128  # 6 contraction chunks
GRP = 16  # tokens per pack group
PACK = GRP * H  # 96 pack rows


def build_graph(n_tok=N_TOK, chunk=512, reps=1, debug=False, bias_zero=False,
                gb=4, eng_rot="vs", tsp_rot="g", mask_eng="g", vq="y", pq="y",
                x0_split=True):
    """Single-core Bass graph (same graph runs SPMD on 8 cores).

    gb: groups per softmax batch (psum tile [96, gb, 96]).
    eng_rot: engine rotation for the QKV psum->sbuf copies.
    tsp_rot: engine rotation for the normalize tensor_scalar.
    vq/pq: queue for vpk/pts transposes ('y'=sync, 's'=scalar).
    x0_split: load chunk 0's x as 4 row-slab DMAs for earlier xT start.
    """
    nc = bacc.Bacc("TRN2", target_bir_lowering=False, debug=debug)

    x_d = nc.dram_tensor("x", [n_tok, C], F32, kind="ExternalInput")
    wqkv_d = nc.dram_tensor("w_qkv", [C, 3 * C], F32, kind="ExternalInput")
    wproj_d = nc.dram_tensor("w_proj", [C, C], F32, kind="ExternalInput")
    bproj_d = nc.dram_tensor("b_proj", [C], F32, kind="ExternalInput")
    m01_d = nc.dram_tensor("m01", [PACK, PACK], BF16, kind="ExternalInput")
    out_d = nc.dram_tensor("out", [n_tok, C], F32, kind="ExternalOutput")

    n_chunks = n_tok // chunk
    ng = chunk // GRP  # pack groups per chunk
    nb = ng // gb  # softmax batches per chunk
    nr = chunk // 128  # 128-token slabs per chunk
    n_tiles = n_tok // 128

    with tile.TileContext(nc) as tc:
        with (
            tc.tile_pool(name="const", bufs=1) as constp,
            tc.tile_pool(name="sb", bufs=2) as sbp,
            tc.tile_pool(name="small", bufs=3) as smallp,
            tc.tile_pool(name="outt", bufs=1) as outtp,
            tc.tile_pool(name="psq", bufs=2, space="PSUM") as psq,
            tc.tile_pool(name="psqk", bufs=2, space="PSUM") as psqk,
            tc.tile_pool(name="psav", bufs=2, space="PSUM") as psav,
            tc.tile_pool(name="pspj", bufs=2, space="PSUM") as pspj,
        ):
            # ---- constants. Casting DMAs are gpsimd-only; x chunk 0 is
            # issued first inside _run_body, weights pace the first QKV
            # accumulation chain, wproj (needed only at proj) goes last.
            wqkv_sb = constp.tile([128, NCH, 3 * C], BF16)
            wproj_sb = constp.tile([128, NCH, C], BF16)
            m01_sb = constp.tile([PACK, PACK], BF16)
            nc.sync.dma_start(out=m01_sb[:], in_=m01_d[:])

            def load_consts():
                # big DMAs (one per q/k/v third): fewer descriptors -> no
                # semaphore-pool pacing stalls
                wqkv_r = wqkv_d[:].rearrange("(ch p) c -> p ch c", p=128)
                for s in range(3):
                    nc.gpsimd.dma_start(
                        out=wqkv_sb[:, :, C * s : C * (s + 1)],
                        in_=wqkv_r[:, :, C * s : C * (s + 1)],
                    )

            def load_wproj():
                nc.gpsimd.dma_start(
                    out=wproj_sb[:],
                    in_=wproj_d[:].rearrange("(j p) c -> p j c", p=128),
                )
            if bias_zero:
                bias_sb = None
            else:
                bias_row = constp.tile([1, C], F32)
                nc.sync.dma_start(out=bias_row[:], in_=bproj_d.ap().unsqueeze(0))
                bias_sb = constp.tile([128, C], F32)
                nc.gpsimd.partition_broadcast(bias_sb[:], bias_row[:])

            # attention output, transposed/mixed: col h*n_tok + t holds
            # out_attn[t, h, :] over the 128 d-partitions. bf16.
            outT = outtp.tile([128, H * n_tok], BF16)

            engs = {"v": nc.vector, "s": nc.scalar, "g": nc.gpsimd}
            rot = [engs[e] for e in eng_rot]
            trot = [engs[e] for e in tsp_rot]

            rot0 = [engs[e] for e in "vs"]

            def copy_op(i, out, in_, first_chunk=False):
                # chunk 0 avoids Pool: its queue is busy with weight DMAs
                r = rot0 if first_chunk else rot
                e = r[i % len(r)]
                if e is nc.scalar:
                    e.copy(out, in_)
                else:
                    e.tensor_copy(out, in_)

            qmap = {"y": nc.sync, "s": nc.scalar}
            for _rep in range(reps):
                _run_body(
                    nc, n_tok, chunk, gb, copy_op, trot, engs[mask_eng],
                    sbp, smallp, psq, psqk, psav, pspj,
                    wqkv_sb, wproj_sb, m01_sb, bias_sb, outT,
                    x_d, out_d,
                    load_consts if _rep == 0 else None,
                    load_wproj if _rep == 0 else None,
                    qmap[vq], qmap[pq], x0_split,
                )

    nc.compile()
    return nc


def _run_body(
    nc, n_tok, chunk, gb, copy_op, trot, mask_eng,
    sbp, smallp, psq, psqk, psav, pspj,
    wqkv_sb, wproj_sb, m01_sb, bias_sb, outT,
    x_d, out_d,
    load_consts=None, load_wproj=None,
    vq_eng=None, pq_eng=None, x0_split=True,
):
    vq_eng = vq_eng or nc.sync
    pq_eng = pq_eng or nc.sync
    n_chunks = n_tok // chunk
    ng = chunk // GRP
    nb = ng // gb
    nr = chunk // 128
    n_tiles = n_tok // 128

    outT_ht = outT[:].rearrange("p (h t) -> p h t", h=H)

    # deferred AV work: (vpk, pts, t0) per chunk, executed one chunk late
    # so the softmax chains have a full chunk of PE work to hide behind.
    pending = []

    def av_phase(vpk, pts, t0):
        for b in range(nb):
            av_ps = psav.tile([128, gb, PACK], F32, tag="av")
            for jj in range(gb):
                j = gb * b + jj
                nc.tensor.matmul(
                    av_ps[:, jj, :],
                    vpk[0:PACK, j, :],
                    pts[0:PACK, j, :],
                    start=True,
                    stop=True,
                )
            tb = t0 + GRP * gb * b
            dst = outT_ht[:, :, tb : tb + GRP * gb].rearrange(
                "p h (j t) -> p j h t", j=gb
            )
            nc.vector.tensor_copy(
                dst, av_ps[:].rearrange("p j (h t) -> p j h t", h=H)
            )

    def load_x(cc, split=False):
        # cast f32->bf16 during DMA; one DMA per chunk in steady state
        x_bf = sbp.tile([128, nr, C], BF16, tag="x_bf")
        if split:
            for rg in range(nr):
                nc.gpsimd.dma_start(
                    out=x_bf[:, rg, :],
                    in_=x_d[cc * chunk + 128 * rg : cc * chunk + 128 * (rg + 1), :],
                )
        else:
            src = x_d[cc * chunk : (cc + 1) * chunk, :].rearrange(
                "(rg p) c -> p rg c", p=128
            )
            nc.gpsimd.dma_start(out=x_bf[:], in_=src)
        return x_bf

    def make_xT(x_bf):
        xT = sbp.tile([128, NCH, chunk], BF16, tag="xT")
        for rg in range(nr):
            for ch in range(NCH):
                nc.sync.dma_start_transpose(
                    out=xT[:, ch, 128 * rg : 128 * (rg + 1)],
                    in_=x_bf[:, rg, 128 * ch : 128 * (ch + 1)],
                )
        return xT

    x_next = load_x(0, split=x0_split)
    xT_next = make_xT(x_next)
    if load_consts is not None:
        load_consts()

    for cc in range(n_chunks):
        t0 = cc * chunk
        xT = xT_next
        if cc == 1 and load_wproj is not None:
            load_wproj()

        # ---- q, k (g-major pack layout), v (group-padded) ----
        # q_il/k_il: [128 d, ng, 96], col inside group = h*16 + t
        # v_ilp:     [128 d, ng, 128], col = g*16 + t, cols 96:128 pad
        q_il = sbp.tile([128, ng, PACK], BF16, tag="q_il")
        k_il = sbp.tile([128, ng, PACK], BF16, tag="k_il")
        v_ilp = sbp.tile([128, ng, 128], BF16, tag="v_ilp")
        q_v = q_il[:].rearrange("p j (s t) -> p j s t", t=GRP)
        k_v = k_il[:].rearrange("p j (s t) -> p j s t", t=GRP)
        v_v = v_ilp[:].rearrange("p j (s t) -> p j s t", t=GRP)
        # pad slots 6,7 feed the xbar transpose (never compute) — zero for
        # the sim's uninit checker
        nc.gpsimd.memset(v_v[:, :, H:8, :], 0.0)
        for s in range(3):
            dst_v = (q_v, k_v, v_v)[s]
            for h in range(H):
                col0 = (s * H + h) * 128
                ps = psq.tile([128, chunk], F32, tag="qkv")
                for ch in range(NCH):
                    nc.tensor.matmul(
                        ps[:],
                        wqkv_sb[:, ch, col0 : col0 + 128],
                        xT[:, ch, :],
                        start=(ch == 0),
                        stop=(ch == NCH - 1),
                    )
                copy_op(
                    s * H + h,
                    dst_v[:, :, h, :],
                    ps[:].rearrange("p (j t) -> p j t", t=GRP),
                    first_chunk=(cc == 0 and load_consts is not None),
                )

        # prefetch next x only now: earlier would block Pool's psum-copy
        # stream behind a 2.4us cast DMA
        if cc + 1 < n_chunks:
            x_next = load_x(cc + 1)
            # next chunk's xT goes on SP *before* this chunk's vpk/pts so
            # it can't get stuck behind them in the queue
            xT_next = make_xT(x_next)

        # ---- V_pack via xbar DMA transpose: rows g*16+t (+pad), cols d ----
        vpk = sbp.tile([128, ng, 128], BF16, tag="vpk")
        for j in range(ng):
            vq_eng.dma_start_transpose(out=vpk[:, j, :], in_=v_ilp[:, j, :])

        # ---- AV for the PREVIOUS chunk, issued before this chunk's
        # softmax so its DVE copies aren't queued behind it ----
        if pending:
            av_phase(*pending.pop())

        # ---- QK^T + softmax + P^T per batch of gb groups ----
        pts = sbp.tile([128, ng, PACK], BF16, tag="pts")
        for b in range(nb):
            qk_ps = psqk.tile([PACK, gb, PACK], F32, tag="qk")
            for jj in range(gb):
                j = gb * b + jj
                nc.tensor.matmul(
                    qk_ps[:, jj, :],
                    q_il[:, j, :],
                    k_il[:, j, :],
                    start=True,
                    stop=True,
                )
            ex = smallp.tile([PACK, gb, PACK], BF16, tag="ex")
            nc.scalar.activation(
                ex[:], qk_ps[:], mybir.ActivationFunctionType.Exp, scale=SCALE
            )
            exm = smallp.tile([PACK, gb, PACK], BF16, tag="exm")
            mask_eng.tensor_tensor(
                exm[:],
                ex[:],
                m01_sb[:].unsqueeze(1).broadcast_to([PACK, gb, PACK]),
                op=mybir.AluOpType.mult,
            )
            zs = smallp.tile([PACK, gb], BF16, tag="zs")
            rc = smallp.tile([PACK, gb], F32, tag="rc")
            with nc.allow_low_precision(reason="softmax denom, 6 terms, bf16 ok"):
                nc.vector.tensor_reduce(
                    zs[:], exm[:], axis=mybir.AxisListType.X, op=mybir.AluOpType.add
                )
                nc.vector.reciprocal(rc[:], zs[:])
            # P padded to 128 cols for the xbar transpose; pad cols hold
            # garbage that lands in unused P^T rows 96:128.
            p_t = smallp.tile([PACK, gb, 128], BF16, tag="p_t")
            nc.gpsimd.memset(p_t[:, :, PACK:128], 0.0)
            for jj in range(gb):
                trot[jj % len(trot)].tensor_scalar(
                    p_t[:, jj, 0:PACK],
                    exm[:, jj, :],
                    rc[:, jj : jj + 1],
                    None,
                    op0=mybir.AluOpType.mult,
                )
            for jj in range(gb):
                j = gb * b + jj
                pq_eng.dma_start_transpose(
                    out=pts[:, j, :], in_=p_t[:, jj, :]
                )

        pending.append((vpk, pts, t0))

    # ---- proj: y[n', o] = sum_j OM[6n'+j] @ Wj (+ b) ----
    # Tiles whose outT columns avoid the last chunk's tokens are issued
    # BEFORE the final AV phase, hiding its softmax latency behind PE work.
    omT = outT[:].rearrange("p (i six) -> p i six", six=H)

    def tile_needs_last_chunk(t):
        return any(
            (col % n_tok) >= n_tok - chunk
            for col in range(768 * t, 768 * (t + 1))
        )

    early = [t for t in range(n_tiles) if not tile_needs_last_chunk(t)]
    late = [t for t in range(n_tiles) if tile_needs_last_chunk(t)]

    def proj_tile(t):
        ya = pspj.tile([128, 384], F32, tag="pj")
        yb = pspj.tile([128, 384], F32, tag="pj")
        for j in range(H):
            lhsT = omT[:, 128 * t : 128 * (t + 1), j]
            nc.tensor.matmul(
                ya[:], lhsT, wproj_sb[:, j, 0:384],
                start=(j == 0), stop=(j == H - 1),
            )
            nc.tensor.matmul(
                yb[:], lhsT, wproj_sb[:, j, 384:768],
                start=(j == 0), stop=(j == H - 1),
            )
        y_sb = smallp.tile([128, C], F32, tag="y_sb")
        if bias_sb is None:
            nc.vector.tensor_copy(y_sb[:, 0:384], ya[:])
            nc.scalar.copy(y_sb[:, 384:768], yb[:])
        else:
            nc.vector.scalar_tensor_tensor(
                y_sb[:, 0:384], ya[:], 1.0, bias_sb[:, 0:384],
                op0=mybir.AluOpType.mult, op1=mybir.AluOpType.add,
            )
            nc.vector.scalar_tensor_tensor(
                y_sb[:, 384:768], yb[:], 1.0, bias_sb[:, 384:768],
                op0=mybir.AluOpType.mult, op1=mybir.AluOpType.add,
            )
        nc.sync.dma_start(out=out_d[128 * t : 128 * (t + 1), :], in_=y_sb[:])

    for t in early:
        proj_tile(t)
    av_phase(*pending.pop())
    for t in late:
        proj_tile(t)


_CACHED = {}

BEST = dict()


def _get_graph(n_tok=N_TOK, chunk=512, **kw):
    key = (n_tok, chunk, tuple(sorted(kw.items())))
    if key not in _CACHED:
        _CACHED[key] = build_graph(n_tok, chunk, **kw)
    return _CACHED[key]


def _m01():
    return np.tile(np.eye(GRP, dtype=ml_dtypes.bfloat16), (H, H))


def make_in_map(x_i, w_qkv, w_proj, b_proj):
    return {
        "x": np.ascontiguousarray(x_i, dtype=np.float32),
        "w_qkv": np.asarray(w_qkv, dtype=np.float32),
        "w_proj": np.asarray(w_proj, dtype=np.float32),
        "b_proj": np.asarray(b_proj, dtype=np.float32),
        "m01": _m01(),
    }


def kernel(x, w_qkv, w_proj, b_proj):
    from concourse.bass_utils import run_bass_kernel_spmd

    x = np.asarray(x, dtype=np.float32)
    nc = _get_graph(
        bias_zero=bool(np.all(np.asarray(b_proj) == 0.0)), **BEST
    )
    in_maps = [make_in_map(x[i], w_qkv, w_proj, b_proj) for i in range(B)]
    res = run_bass_kernel_spmd(nc, in_maps, core_ids=list(range(B)))
    out = np.stack([res.results[i]["out"] for i in range(B)], axis=0)
    return out.astype(np.float32)


if __name__ == "__main__":
    rng = np.random.default_rng(0)
    x = rng.standard_normal((B, N_TOK, C), dtype=np.float32)
    w_qkv = (rng.standard_normal((C, 3 * C)) * C**-0.5).astype(np.float32)
    w_proj = (rng.standard_normal((C, C)) * C**-0.5).astype(np.float32)
    b_proj = np.zeros((C,), dtype=np.float32)
    y = kernel(x=x, w_qkv=w_qkv, w_proj=w_proj, b_proj=b_proj)
    print(y.shape, y.dtype)


# revision 51
# speedup vs baseline: 2817.0024x; 1.8145x over previous
"""Trainium2 Bass kernel for nn_Attention_19035295056566 (v2).

Dense transformer block with head-axis attention: per token a 6x6
softmax over the head axis; torch-faithful dim-mixing transpose; then
proj. Pack-16 formulation: 16 tokens x 6 heads = 96 rows per PE
matmul, g-major packing (pack row = g*16 + t), DMA-transpose (xbar)
for every layout move (no PE transposes, no DRAM bounce), block-diag
mask applied multiplicatively after exp.

Distribution: pure data-parallel over batch B=8 across 8 NeuronCores
(one batch element per core, weights replicated, no collectives).

Self-contained: hardcodes shapes B=8, N=4096, C=768, H=6, D=128.
"""

import sys

for _p in ("/opt/trn_rl_repo",):
    if _p not in sys.path:
        sys.path.insert(0, _p)

import numpy as np
import ml_dtypes

from concourse import bass, bacc, mybir, tile

F32 = mybir.dt.float32
BF16 = mybir.dt.bfloat16

B, N_TOK, C = 8, 4096, 768
H, D = 6, 128
SCALE = float(D) ** -0.5
NCH = C // 128  # 6 contraction chunks
GRP = 16  # tokens per pack group
PACK = GRP * H  # 96 pack rows


def build_graph(n_tok=N_TOK, chunk=512, reps=1, debug=False, bias_zero=False,
                gb=4, eng_rot="vs", tsp_rot="g", mask_eng="g", vq="y", pq="y",
                x0_split=True):
    """Single-core Bass graph (same graph runs SPMD on 8 cores).

    gb: groups per softmax batch (psum tile [96, gb, 96]).
    eng_rot: engine rotation for the QKV psum->sbuf copies.
    tsp_rot: engine rotation for the normalize tensor_scalar.
    vq/pq: queue for vpk/pts transposes ('y'=sync, 's'=scalar).
    x0_split: load chunk 0's x as 4 row-slab DMAs for earlier xT start.
    """
    nc = bacc.Bacc("TRN2", target_bir_lowering=False, debug=debug)

    x_d = nc.dram_tensor("x", [n_tok, C], F32, kind="ExternalInput")
    wqkv_d = nc.dram_tensor("w_qkv", [C, 3 * C], F32, kind="ExternalInput")
    wproj_d = nc.dram_tensor("w_proj", [C, C], F32, kind="ExternalInput")
    bproj_d = nc.dram_tensor("b_proj", [C], F32, kind="ExternalInput")
    m01_d = nc.dram_tensor("m01", [PACK, PACK], BF16, kind="ExternalInput")
    out_d = nc.dram_tensor("out", [n_tok, C], F32, kind="ExternalOutput")

    n_chunks = n_tok // chunk
    ng = chunk // GRP  # pack groups per chunk
    nb = ng // gb  # softmax batches per chunk
    nr = chunk // 128  # 128-token slabs per chunk
    n_tiles = n_tok // 128

    with tile.TileContext(nc) as tc:
        with (
            tc.tile_pool(name="const", bufs=1) as constp,
            tc.tile_pool(name="sb", bufs=2) as sbp,
            tc.tile_pool(name="small", bufs=3) as smallp,
            tc.tile_pool(name="outt", bufs=1) as outtp,
            tc.tile_pool(name="psq", bufs=2, space="PSUM") as psq,
            tc.tile_pool(name="psqk", bufs=2, space="PSUM") as psqk,
            tc.tile_pool(name="psav", bufs=2, space="PSUM") as psav,
            tc.tile_pool(name="pspj", bufs=2, space="PSUM") as pspj,
        ):
            # ---- constants. Casting DMAs are gpsimd-only; x chunk 0 is
            # issued first inside _run_body, weights pace the first QKV
            # accumulation chain, wproj (needed only at proj) goes last.
            wqkv_sb = constp.tile([128, NCH, 3 * C], BF16)
            wproj_sb = constp.tile([128, NCH, C], BF16)
            m01_sb = constp.tile([PACK, PACK], BF16)
            nc.sync.dma_start(out=m01_sb[:], in_=m01_d[:])

            def load_consts():
                # big DMAs (one per q/k/v third): fewer descriptors -> no
                # semaphore-pool pacing stalls
                wqkv_r = wqkv_d[:].rearrange("(ch p) c -> p ch c", p=128)
                for s in range(3):
                    nc.gpsimd.dma_start(
                        out=wqkv_sb[:, :, C * s : C * (s + 1)],
                        in_=wqkv_r[:, :, C * s : C * (s + 1)],
                    )

            def load_wproj():
                nc.gpsimd.dma_start(
                    out=wproj_sb[:],
                    in_=wproj_d[:].rearrange("(j p) c -> p j c", p=128),
                )
            if bias_zero:
                bias_sb = None
            else:
                bias_row = constp.tile([1, C], F32)
                nc.sync.dma_start(out=bias_row[:], in_=bproj_d.ap().unsqueeze(0))
                bias_sb = constp.tile([128, C], F32)
                nc.gpsimd.partition_broadcast(bias_sb[:], bias_row[:])

            # attention output, transposed/mixed: col h*n_tok + t holds
            # out_attn[t, h, :] over the 128 d-partitions. bf16.
            outT = outtp.tile([128, H * n_tok], BF16)

            engs = {"v": nc.vector, "s": nc.scalar, "g": nc.gpsimd}
            rot = [engs[e] for e in eng_rot]
            trot = [engs[e] for e in tsp_rot]

            rot0 = [engs[e] for e in "vs"]

            def copy_op(i, out, in_, first_chunk=False):
                # chunk 0 avoids Pool: its queue is busy with weight DMAs
                r = rot0 if first_chunk else rot
                e = r[i % len(r)]
                if e is nc.scalar:
                    e.copy(out, in_)
                else:
                    e.tensor_copy(out, in_)

            qmap = {"y": nc.sync, "s": nc.scalar}
            for _rep in range(reps):
                _run_body(
                    nc, n_tok, chunk, gb, copy_op, trot, engs[mask_eng],
                    sbp, smallp, psq, psqk, psav, pspj,
                    wqkv_sb, wproj_sb, m01_sb, bias_sb, outT,
                    x_d, out_d,
                    load_consts if _rep == 0 else None,
                    load_wproj if _rep == 0 else None,
                    qmap[vq], qmap[pq], x0_split,
                )

    nc.compile()
    return nc


def _run_body(
    nc, n_tok, chunk, gb, copy_op, trot, mask_eng,
    sbp, smallp, psq, psqk, psav, pspj,
    wqkv_sb, wproj_sb, m01_sb, bias_sb, outT,
    x_d, out_d,
    load_consts=None, load_wproj=None,
    vq_eng=None, pq_eng=None, x0_split=False,
):
    vq_eng = vq_eng or nc.sync
    pq_eng = pq_eng or nc.sync
    n_chunks = n_tok // chunk
    ng = chunk // GRP
    nb = ng // gb
    nr = chunk // 128
    n_tiles = n_tok // 128

    outT_ht = outT[:].rearrange("p (h t) -> p h t", h=H)

    # deferred AV work: (vpk, pts, t0) per chunk, executed one chunk late
    # so the softmax chains have a full chunk of PE work to hide behind.
    pending = []

    def av_phase(vpk, pts, t0):
        for b in range(nb):
            av_ps = psav.tile([128, gb, PACK], F32, tag="av")
            for jj in range(gb):
                j = gb * b + jj
                nc.tensor.matmul(
                    av_ps[:, jj, :],
                    vpk[0:PACK, j, :],
                    pts[0:PACK, j, :],
                    start=True,
                    stop=True,
                )
            tb = t0 + GRP * gb * b
            dst = outT_ht[:, :, tb : tb + GRP * gb].rearrange(
                "p h (j t) -> p j h t", j=gb
            )
            nc.vector.tensor_copy(
                dst, av_ps[:].rearrange("p j (h t) -> p j h t", h=H)
            )

    def load_x(cc, split=False):
        # cast f32->bf16 during DMA; one DMA per chunk in steady state
        x_bf = sbp.tile([128, nr, C], BF16, tag="x_bf")
        if split:
            for rg in range(nr):
                nc.gpsimd.dma_start(
                    out=x_bf[:, rg, :],
                    in_=x_d[cc * chunk + 128 * rg : cc * chunk + 128 * (rg + 1), :],
                )
        else:
            src = x_d[cc * chunk : (cc + 1) * chunk, :].rearrange(
                "(rg p) c -> p rg c", p=128
            )
            nc.gpsimd.dma_start(out=x_bf[:], in_=src)
        return x_bf

    def make_xT(x_bf):
        # one batched xbar transpose: block j = rg*NCH+ch gets
        # x_bf[:, rg, 128ch:128(ch+1)]^T  ->  [c-partitions, 128 tokens]
        xT = sbp.tile([128, nr * NCH, 128], BF16, tag="xT")
        nc.sync.dma_start_transpose(out=xT[:], in_=x_bf[:])
        return xT[:].rearrange("p (rg ch) t -> p ch rg t", ch=NCH)

    x_next = load_x(0, split=x0_split)
    xT_next = make_xT(x_next)
    if load_consts is not None:
        load_consts()

    for cc in range(n_chunks):
        t0 = cc * chunk
        xT = xT_next
        if cc == 1 and load_wproj is not None:
            load_wproj()

        # ---- q, k (g-major pack layout), v (group-padded) ----
        # q_il/k_il: [128 d, ng, 96], col inside group = h*16 + t
        # v_ilp:     [128 d, ng, 128], col = g*16 + t, cols 96:128 pad
        q_il = sbp.tile([128, ng, PACK], BF16, tag="q_il")
        k_il = sbp.tile([128, ng, PACK], BF16, tag="k_il")
        v_ilp = sbp.tile([128, ng, 128], BF16, tag="v_ilp")
        q_v = q_il[:].rearrange("p j (s t) -> p j s t", t=GRP)
        k_v = k_il[:].rearrange("p j (s t) -> p j s t", t=GRP)
        v_v = v_ilp[:].rearrange("p j (s t) -> p j s t", t=GRP)
        # pad slots 6,7 feed the xbar transpose (never compute) — zero for
        # the sim's uninit checker
        nc.gpsimd.memset(v_v[:, :, H:8, :], 0.0)
        for s in range(3):
            dst_v = (q_v, k_v, v_v)[s]
            for h in range(H):
                col0 = (s * H + h) * 128
                ps = psq.tile([128, chunk], F32, tag="qkv")
                for ch in range(NCH):
                    nc.tensor.matmul(
                        ps[:],
                        wqkv_sb[:, ch, col0 : col0 + 128],
                        xT[:, ch, :, :],
                        start=(ch == 0),
                        stop=(ch == NCH - 1),
                    )
                copy_op(
                    s * H + h,
                    dst_v[:, :, h, :],
                    ps[:].rearrange("p (j t) -> p j t", t=GRP),
                    first_chunk=(cc == 0 and load_consts is not None),
                )

        # prefetch next x only now: earlier would block Pool's psum-copy
        # stream behind a 2.4us cast DMA
        if cc + 1 < n_chunks:
            x_next = load_x(cc + 1)
            # next chunk's xT goes on SP *before* this chunk's vpk/pts so
            # it can't get stuck behind them in the queue
            xT_next = make_xT(x_next)

        # ---- V_pack via one batched xbar transpose: block j gets
        # v_ilp[:, j, :]^T = [(g*16+t) rows (+pad), d cols] ----
        vpk = sbp.tile([128, ng, 128], BF16, tag="vpk")
        vq_eng.dma_start_transpose(out=vpk[:], in_=v_ilp[:])

        # ---- AV for the PREVIOUS chunk, issued before this chunk's
        # softmax so its DVE copies aren't queued behind it ----
        if pending:
            av_phase(*pending.pop())

        # ---- QK^T + softmax + P^T per batch of gb groups ----
        pts = sbp.tile([128, ng, PACK], BF16, tag="pts")
        for b in range(nb):
            qk_ps = psqk.tile([PACK, gb, PACK], F32, tag="qk")
            for jj in range(gb):
                j = gb * b + jj
                nc.tensor.matmul(
                    qk_ps[:, jj, :],
                    q_il[:, j, :],
                    k_il[:, j, :],
                    start=True,
                    stop=True,
                )
            ex = smallp.tile([PACK, gb, PACK], BF16, tag="ex")
            nc.scalar.activation(
                ex[:], qk_ps[:], mybir.ActivationFunctionType.Exp, scale=SCALE
            )
            exm = smallp.tile([PACK, gb, PACK], BF16, tag="exm")
            mask_eng.tensor_tensor(
                exm[:],
                ex[:],
                m01_sb[:].unsqueeze(1).broadcast_to([PACK, gb, PACK]),
                op=mybir.AluOpType.mult,
            )
            zs = smallp.tile([PACK, gb], BF16, tag="zs")
            rc = smallp.tile([PACK, gb], F32, tag="rc")
            with nc.allow_low_precision(reason="softmax denom, 6 terms, bf16 ok"):
                nc.vector.tensor_reduce(
                    zs[:], exm[:], axis=mybir.AxisListType.X, op=mybir.AluOpType.add
                )
                nc.vector.reciprocal(rc[:], zs[:])
            # P padded to 128 cols for the xbar transpose; pad cols hold
            # garbage that lands in unused P^T rows 96:128.
            p_t = smallp.tile([PACK, gb, 128], BF16, tag="p_t")
            nc.gpsimd.memset(p_t[:, :, PACK:128], 0.0)
            for jj in range(gb):
                trot[jj % len(trot)].tensor_scalar(
                    p_t[:, jj, 0:PACK],
                    exm[:, jj, :],
                    rc[:, jj : jj + 1],
                    None,
                    op0=mybir.AluOpType.mult,
                )
            pq_eng.dma_start_transpose(
                out=pts[:, gb * b : gb * (b + 1), :], in_=p_t[:]
            )

        pending.append((vpk, pts, t0))

    # ---- proj: y[n', o] = sum_j OM[6n'+j] @ Wj (+ b) ----
    # Tiles whose outT columns avoid the last chunk's tokens are issued
    # BEFORE the final AV phase, hiding its softmax latency behind PE work.
    omT = outT[:].rearrange("p (i six) -> p i six", six=H)

    def tile_needs_last_chunk(t):
        return any(
            (col % n_tok) >= n_tok - chunk
            for col in range(768 * t, 768 * (t + 1))
        )

    early = [t for t in range(n_tiles) if not tile_needs_last_chunk(t)]
    late = [t for t in range(n_tiles) if tile_needs_last_chunk(t)]

    def proj_tile(t):
        ya = pspj.tile([128, 384], F32, tag="pj")
        yb = pspj.tile([128, 384], F32, tag="pj")
        for j in range(H):
            lhsT = omT[:, 128 * t : 128 * (t + 1), j]
            nc.tensor.matmul(
                ya[:], lhsT, wproj_sb[:, j, 0:384],
                start=(j == 0), stop=(j == H - 1),
            )
            nc.tensor.matmul(
                yb[:], lhsT, wproj_sb[:, j, 384:768],
                start=(j == 0), stop=(j == H - 1),
            )
        y_sb = smallp.tile([128, C], F32, tag="y_sb")
        if bias_sb is None:
            nc.vector.tensor_copy(y_sb[:, 0:384], ya[:])
            nc.scalar.copy(y_sb[:, 384:768], yb[:])
        else:
            nc.vector.scalar_tensor_tensor(
                y_sb[:, 0:384], ya[:], 1.0, bias_sb[:, 0:384],
                op0=mybir.AluOpType.mult, op1=mybir.AluOpType.add,
            )
            nc.vector.scalar_tensor_tensor(
                y_sb[:, 384:768], yb[:], 1.0, bias_sb[:, 384:768],
                op0=mybir.AluOpType.mult, op1=mybir.AluOpType.add,
            )
        (nc.sync if t % 2 == 0 else nc.scalar).dma_start(
            out=out_d[128 * t : 128 * (t + 1), :], in_=y_sb[:]
        )

    for t in early:
        proj_tile(t)
    av_phase(*pending.pop())
    for t in late:
        proj_tile(t)


_CACHED = {}

BEST = dict()


def _get_graph(n_tok=N_TOK, chunk=512, **kw):
    key = (n_tok, chunk, tuple(sorted(kw.items())))
    if key not in _CACHED:
        _CACHED[key] = build_graph(n_tok, chunk, **kw)
    return _CACHED[key]


def _m01():
    return np.tile(np.eye(GRP, dtype=ml_dtypes.bfloat16), (H, H))


def make_in_map(x_i, w_qkv, w_proj, b_proj):
    return {
        "x": np.ascontiguousarray(x_i, dtype=np.float32),
        "w_qkv": np.asarray(w_qkv, dtype=np.float32),
        "w_proj": np.asarray(w_proj, dtype=np.float32),
        "b_proj": np.asarray(b_proj, dtype=np.float32),
        "m01": _m01(),
    }


def kernel(x, w_qkv, w_proj, b_proj):
    from concourse.bass_utils import run_bass_kernel_spmd

    x = np.asarray(x, dtype=np.float32)
    nc = _get_graph(
        bias_zero=bool(np.all(np.asarray(b_proj) == 0.0)), **BEST
    )
    in_maps = [make_in_map(x[i], w_qkv, w_proj, b_proj) for i in range(B)]
    res = run_bass_kernel_spmd(nc, in_maps, core_ids=list(range(B)))
    out = np.stack([res.results[i]["out"] for i in range(B)], axis=0)
    return out.astype(np.float32)


if __name__ == "__main__":
    rng = np.random.default_rng(0)
    x = rng.standard_normal((B, N_TOK, C), dtype=np.float32)
    w_qkv = (rng.standard_normal((C, 3 * C)) * C**-0.5).astype(np.float32)
    w_proj = (rng.standard_normal((C, C)) * C**-0.5).astype(np.float32)
    b_proj = np.zeros((C,), dtype=np.float32)
    y = kernel(x=x, w_qkv=w_qkv, w_proj=w_proj, b_proj=b_proj)
    print(y.shape, y.dtype)
